# revision 2
# baseline (speedup 1.0000x reference)
"""AttnCutLoss on 8 Trainium2 NeuronCores (pure data parallel over batch).

loss = -sum_{b,j} log(output[b,j]) * q[b,j] / B,  q = softmax_j(r/tau),
r[b,j] = F1-at-cutoff-(j+1) = 2*csum[b,j] / (j+1 + T[b])   (harmonic-mean
identity; exact also when csum==0 or T==0), with csum = cumsum_j(labels),
T = total relevant per row.

z = r/tau lies in [0, 2/(2*tau)] ⊂ [0, 1.06], so softmax needs no
max-subtraction: per row, loss_b = -sum(ln(out)*e^z)/sum(e^z).

Per-core pipeline over 8 tiles of [128 rows, 2048]:
  DMA labels,out -> scan (cumsum, DVE) -> denom=k+T (DVE tensor_scalar)
  -> recip (DVE approx) -> z=(2/tau)*csum*recip (DVE STT)
  -> e=exp(z), s=sum(e) (ACT accum) -> lo=ln(out) (ACT)
  -> ip=sum(e*lo) (DVE tensor_tensor_reduce)
Device returns per-row partial sums ip,s; host computes -sum(ip/s)/B.
"""

import numpy as np

import concourse.bass as bass
import concourse.tile as tile
from concourse import bacc, mybir
from concourse.bass_utils import run_bass_kernel_spmd

B, L = 8192, 2048
N_CORES = 8
ROWS_PER_CORE = B // N_CORES          # 1024
P = 128                               # SBUF partitions
TILES_PER_CORE = ROWS_PER_CORE // P   # 8
TAU = 0.95

_NC_CACHE = {}


def _build_nc():
    f32 = mybir.dt.float32
    AF = mybir.ActivationFunctionType
    OP = mybir.AluOpType

    nc = bacc.Bacc("TRN2", target_bir_lowering=False, debug=False)
    labels_d = nc.dram_tensor("labels", [ROWS_PER_CORE, L], f32, kind="ExternalInput")
    outp_d = nc.dram_tensor("outp", [ROWS_PER_CORE, L], f32, kind="ExternalInput")
    kvec_d = nc.dram_tensor("kvec", [P, L], f32, kind="ExternalInput")
    ip_d = nc.dram_tensor("ip_out", [P, TILES_PER_CORE], f32, kind="ExternalOutput")
    s_d = nc.dram_tensor("s_out", [P, TILES_PER_CORE], f32, kind="ExternalOutput")

    with tile.TileContext(nc) as tc:
        with (
            tc.tile_pool(name="const", bufs=1) as cpool,
            tc.tile_pool(name="io", bufs=3) as iopool,
            tc.tile_pool(name="work", bufs=2) as wpool,
            tc.tile_pool(name="res", bufs=1) as rpool,
        ):
            kt = cpool.tile([P, L], f32)
            nc.sync.dma_start(kt[:], kvec_d.ap())
            ip_sb = rpool.tile([P, TILES_PER_CORE], f32)
            s_sb = rpool.tile([P, TILES_PER_CORE], f32)

            for t in range(TILES_PER_CORE):
                rows = slice(t * P, (t + 1) * P)
                lab = iopool.tile([P, L], f32)
                nc.sync.dma_start(lab[:], labels_d.ap()[rows, :])
                out = iopool.tile([P, L], f32)
                nc.sync.dma_start(out[:], outp_d.ap()[rows, :])

                # cumsum along the row
                csum = wpool.tile([P, L], f32)
                nc.vector.tensor_tensor_scan(
                    csum[:], lab[:], lab[:], 0.0, OP.add, OP.bypass
                )
                # denom = k + T  (T = csum[:, -1])
                denom = wpool.tile([P, L], f32)
                nc.vector.tensor_scalar_add(denom[:], kt[:], csum[:, L - 1 : L])
                recip = wpool.tile([P, L], f32)
                nc.vector.reciprocal_approx_fast(out=recip[:], in_=denom[:])
                # z = (2/tau) * csum * recip
                z = wpool.tile([P, L], f32)
                nc.vector.scalar_tensor_tensor(
                    z[:], csum[:], 2.0 / TAU, recip[:], OP.mult, OP.mult
                )
                # e = exp(z), s = sum(e)
                e = wpool.tile([P, L], f32)
                nc.scalar.activation(
                    e[:], z[:], AF.Exp, accum_out=s_sb[:, t : t + 1]
                )
                # lo = ln(out)
                lo = wpool.tile([P, L], f32)
                nc.scalar.activation(lo[:], out[:], AF.Ln)
                # ip = sum(e * lo)   (tensor_tensor_reduce crashes TRN2 here;
                # scalar_tensor_tensor's accumulator does the same fusion)
                w = wpool.tile([P, L], f32)
                nc.vector.scalar_tensor_tensor(
                    w[:],
                    e[:],
                    0.0,
                    lo[:],
                    OP.bypass,
                    OP.mult,
                    accum_out=ip_sb[:, t : t + 1],
                )

            nc.sync.dma_start(ip_d.ap(), ip_sb[:])
            nc.sync.dma_start(s_d.ap(), s_sb[:])
    nc.compile()
    return nc


def _get_nc():
    if "nc" not in _NC_CACHE:
        _NC_CACHE["nc"] = _build_nc()
    return _NC_CACHE["nc"]


def _make_in_maps(output, labels):
    outp = np.ascontiguousarray(
        np.asarray(output, dtype=np.float32).reshape(B, L)
    )
    lab = np.ascontiguousarray(np.asarray(labels, dtype=np.float32))
    kvec = np.ascontiguousarray(
        np.broadcast_to(np.arange(1, L + 1, dtype=np.float32), (P, L))
    )
    in_maps = []
    for c in range(N_CORES):
        rows = slice(c * ROWS_PER_CORE, (c + 1) * ROWS_PER_CORE)
        in_maps.append({"labels": lab[rows], "outp": outp[rows], "kvec": kvec})
    return in_maps


def _reduce_results(results):
    total = 0.0
    for r in results:
        ip = r["ip_out"].astype(np.float64)
        s = r["s_out"].astype(np.float64)
        total += float((ip / s).sum())
    return np.float32(-total / B)


def kernel(output, labels):
    nc = _get_nc()
    in_maps = _make_in_maps(output, labels)
    res = run_bass_kernel_spmd(nc, in_maps, list(range(N_CORES)))
    return _reduce_results(res.results)


# revision 3
# speedup vs baseline: 1.0489x; 1.0489x over previous
"""AttnCutLoss on 8 Trainium2 NeuronCores (pure data parallel over batch).

loss = -sum_{b,j} log(output[b,j]) * q[b,j] / B,  q = softmax_j(r/tau),
r[b,j] = F1-at-cutoff-(j+1) = 2*csum[b,j] / (j+1 + T[b])   (harmonic-mean
identity; exact also when csum==0 or T==0), with csum = cumsum_j(labels),
T = total relevant per row.

z = r/tau lies in [0, 1/tau] ⊂ [0, 1.06], so softmax needs no
max-subtraction: per row, loss_b = -sum(ln(out)*e^z)/sum(e^z).

Device mapping (per core, 8 tiles of [128 rows x 2048]):
  labels ship as uint8 (0/1, lossless), output as float16 (~5e-4 rel,
  washes out in the 16M-term average).  The per-element 1/(k+T[b]) factor
  comes from a host-built constant table RTAB[T, j] = (2/tau)/(j+1+T)
  (float16, [2049, 2048]) fetched per tile with an indirect row-gather
  keyed by T - this removes reciprocal work from the device entirely.

  DVE : cumsum scan (u8 -> f16), ip = sum(e*lo) via fused STT-accumulate
  Pool: T -> int32 offsets, indirect gather, z = csum * recip
  ACT : e = exp(z) with s = sum(e) accumulator, lo = ln(out)
Host: loss = -sum(ip/s)/B in float64, cast to float32.

The Bacc activation-table pass is pinned so Exp and Ln share one table
(natural_log_exp_and_others); the default greedy choice alternates two
tables and pays a 1.3us ACT_TABLE_LOAD per activation.
"""

import numpy as np

import bass_rust as _bass_rust
import concourse.bass as bass
import concourse.tile as tile
from concourse import bacc, mybir
from concourse.bass_utils import run_bass_kernel_spmd
from concourse.hw_specs import get_activation_tables

B, L = 8192, 2048
N_CORES = 8
ROWS_PER_CORE = B // N_CORES          # 1024
P = 128                               # SBUF partitions
TILES_PER_CORE = ROWS_PER_CORE // P   # 8
TAU = 0.95
VTAB = L + 1                          # T can be 0..2048

_CACHE = {}


def _pin_act_tables(nc):
    """Per-instance override: keep Exp/Ln only in the combined table so the
    table-load pass can't alternate between the exp-only and ln-only sets."""

    def patched(self):
        has_activation = any(
            isinstance(i, mybir.InstActivation)
            for b in self.main_func.blocks
            for i in b.instructions
        )
        if not has_activation:
            return
        AF = mybir.ActivationFunctionType
        keep = "natural_log_exp_and_others"
        tables = []
        for name, funcs in get_activation_tables(self.m.arch).items():
            if name != keep:
                funcs = {f for f in funcs if f not in (AF.Exp, AF.Ln)}
            tables.append((name, funcs))
        _bass_rust.insert_act_table_loads(self, tables)

    nc.insert_act_table_loads = patched.__get__(nc)


def _build_nc():
    f16 = mybir.dt.float16
    f32 = mybir.dt.float32
    i32 = mybir.dt.int32
    u8 = mybir.dt.uint8
    AF = mybir.ActivationFunctionType
    OP = mybir.AluOpType

    nc = bacc.Bacc("TRN2", target_bir_lowering=False, debug=False)
    _pin_act_tables(nc)
    labels_d = nc.dram_tensor("labels", [ROWS_PER_CORE, L], u8, kind="ExternalInput")
    outp_d = nc.dram_tensor("outp", [ROWS_PER_CORE, L], f16, kind="ExternalInput")
    rtab_d = nc.dram_tensor("rtab", [VTAB, L], f16, kind="ExternalInput")
    ip_d = nc.dram_tensor("ip_out", [P, TILES_PER_CORE], f32, kind="ExternalOutput")
    s_d = nc.dram_tensor("s_out", [P, TILES_PER_CORE], f32, kind="ExternalOutput")

    with tile.TileContext(nc) as tc:
        with (
            tc.tile_pool(name="io", bufs=3) as iopool,
            tc.tile_pool(name="work", bufs=2) as wpool,
            tc.tile_pool(name="res", bufs=1) as rpool,
        ):
            ip_sb = rpool.tile([P, TILES_PER_CORE], f32)
            s_sb = rpool.tile([P, TILES_PER_CORE], f32)

            for t in range(TILES_PER_CORE):
                rows = slice(t * P, (t + 1) * P)
                lab = iopool.tile([P, L], u8)
                nc.sync.dma_start(lab[:], labels_d.ap()[rows, :])
                out = iopool.tile([P, L], f16)
                nc.sync.dma_start(out[:], outp_d.ap()[rows, :])

                # cumsum along the row (u8 in, f16 out: integers <= 2048, exact)
                csum = wpool.tile([P, L], f16)
                nc.vector.tensor_tensor_scan(
                    csum[:], lab[:], lab[:], 0.0, OP.add, OP.bypass
                )
                # T = csum[:, -1] as int32 row index into the reciprocal table
                offs = wpool.tile([P, 1], i32)
                nc.gpsimd.tensor_copy(offs[:], csum[:, L - 1 : L])
                recip = wpool.tile([P, L], f16)
                nc.gpsimd.indirect_dma_start(
                    out=recip[:],
                    out_offset=None,
                    in_=rtab_d.ap(),
                    in_offset=bass.IndirectOffsetOnAxis(ap=offs[:, :1], axis=0),
                )
                # z = (2/tau) * csum / (k + T)
                z = wpool.tile([P, L], f16)
                nc.gpsimd.tensor_tensor(out=z[:], in0=csum[:], in1=recip[:], op=OP.mult)
                # e = exp(z), s = sum(e)
                e = wpool.tile([P, L], f16)
                nc.scalar.activation(e[:], z[:], AF.Exp, accum_out=s_sb[:, t : t + 1])
                # lo = ln(out)
                lo = wpool.tile([P, L], f16)
                nc.scalar.activation(lo[:], out[:], AF.Ln)
                # ip = sum(e * lo), fused multiply + accumulate on DVE
                w = wpool.tile([P, L], f16)
                nc.vector.scalar_tensor_tensor(
                    w[:], e[:], 0.0, lo[:], OP.bypass, OP.mult,
                    accum_out=ip_sb[:, t : t + 1],
                )

            nc.sync.dma_start(ip_d.ap(), ip_sb[:])
            nc.sync.dma_start(s_d.ap(), s_sb[:])
    nc.compile()
    return nc


def _get_nc():
    if "nc" not in _CACHE:
        _CACHE["nc"] = _build_nc()
    return _CACHE["nc"]


def _get_rtab():
    if "rtab" not in _CACHE:
        t = np.arange(VTAB, dtype=np.float64)[:, None]
        k = np.arange(1, L + 1, dtype=np.float64)[None, :]
        _CACHE["rtab"] = ((2.0 / TAU) / (k + t)).astype(np.float16)
    return _CACHE["rtab"]


def _make_in_maps(output, labels):
    outp = np.asarray(output, dtype=np.float32).reshape(B, L).astype(np.float16)
    lab = np.asarray(labels).astype(np.uint8)
    rtab = _get_rtab()
    in_maps = []
    for c in range(N_CORES):
        rows = slice(c * ROWS_PER_CORE, (c + 1) * ROWS_PER_CORE)
        in_maps.append(
            {
                "labels": np.ascontiguousarray(lab[rows]),
                "outp": np.ascontiguousarray(outp[rows]),
                "rtab": rtab,
            }
        )
    return in_maps


def _reduce_results(results):
    total = 0.0
    for r in results:
        ip = r["ip_out"].astype(np.float64)
        s = r["s_out"].astype(np.float64)
        total += float((ip / s).sum())
    return np.float32(-total / B)


def kernel(output, labels):
    nc = _get_nc()
    in_maps = _make_in_maps(output, labels)
    res = run_bass_kernel_spmd(nc, in_maps, list(range(N_CORES)))
    return _reduce_results(res.results)
